# revision 4
# baseline (speedup 1.0000x reference)
"""GQA (B=1, S=2048, D=4096, H=32, G=8) on 8 TRN2 NeuronCores.

Sharding: tensor-parallel over heads — core c owns query heads 4c..4c+3 and
KV group c. Per core: qT/kT/vT projections from full x (transposed layouts),
RoPE, causal attention with transposed probs (exp without max-subtraction —
scores are bounded; denominator via a ones-column in V'), normalization folded
into a per-partition scalar multiply, PE-transpose of ctx, AllGather of ctxT
(4MB/rank), then a column-sharded Wo matmul. Host slices weights / transposes
x / concatenates output columns.

Self-contained: no sibling imports; hardcoded shapes.
"""
import contextlib
import ctypes
import os
import sys
import types

import numpy as np

os.environ.setdefault("MYCRO_LOCAL_CACHE", "1")

for _p in ("/opt/trn_rl_repo", "/root/.axon_site/_ro/trn_rl_repo"):
    if _p not in sys.path and os.path.isdir(_p):
        sys.path.append(_p)

import concourse.bass as bass
import concourse.tile as tile
from concourse import mybir
from concourse.bass_utils import run_bass_kernel_spmd
from concourse.masks import make_identity

# ---------------------------------------------------------------- profiling shim
_SO_PATH = "/opt/axon/libaxon_pjrt.so"
_hook_holder = [None]


def _ntff_profile_via_ctypes(so_path):
    try:
        lib = ctypes.CDLL(so_path)
    except OSError:
        return None
    if not hasattr(lib, "axon_start_nrt_profile"):
        return None
    lib.axon_start_nrt_profile.argtypes = [
        ctypes.POINTER(ctypes.c_int64),
        ctypes.c_size_t,
    ]
    lib.axon_start_nrt_profile.restype = ctypes.c_int64
    lib.axon_stop_nrt_profile.argtypes = [ctypes.c_char_p]
    lib.axon_stop_nrt_profile.restype = ctypes.c_int64

    @contextlib.contextmanager
    def _hook(output_dir, device_ids):
        import jax

        jax.devices()
        if device_ids:
            ids = (ctypes.c_int64 * len(device_ids))(*device_ids)
            rc = lib.axon_start_nrt_profile(ids, len(device_ids))
        else:
            rc = lib.axon_start_nrt_profile(None, 0)
        if rc != 0:
            raise RuntimeError(f"axon_start_nrt_profile rc={rc}")
        try:
            yield
        finally:
            n = lib.axon_stop_nrt_profile(str(output_dir).encode())
            if n <= 0:
                print(f"WARNING: ntff capture wrote {n} files", file=sys.stderr)

    return _hook


def _install_prof_shim():
    if "antenv.axon_hooks" not in sys.modules:
        mod = types.ModuleType("antenv.axon_hooks")
        mod.set_axon_ntff_profile_hook = lambda h: _hook_holder.__setitem__(0, h)
        mod.get_axon_ntff_profile_hook = lambda: _hook_holder[0]
        sys.modules["antenv.axon_hooks"] = mod
    _hook_holder[0] = _ntff_profile_via_ctypes(_SO_PATH)
    import concourse.bass_utils as bu

    bu.upload_artifacts = lambda tmpdir: tmpdir


_install_prof_shim()

# ------------------------------------------------------------- wait-split pass
def _split_multi_waits(nc, maxw=1):
    """walrus in this container allows only one sync-wait per instruction;
    split extras onto nops inserted before the offender (same engine/block)."""

    def _remove_by_name(name):
        for f in nc.m.functions:
            for bb in f.blocks:
                for i, inst in enumerate(bb.instructions):
                    if inst.name == name:
                        lst = bb.instructions
                        del lst[i]
                        bb.instructions = lst
                        return inst
        raise KeyError(name)

    offenders = []
    for f in nc.m.functions:
        for bb in f.blocks:
            for inst in bb.instructions:
                si = inst.sync_info
                if si and si.on_wait and len(si.on_wait) > maxw:
                    offenders.append(inst.name)
    for name in offenders:
        target = None
        for f in nc.m.functions:
            for bb in f.blocks:
                for idx, inst in enumerate(bb.instructions):
                    if inst.name == name:
                        target = (bb, inst)
                        break
                if target:
                    break
            if target:
                break
        bb, inst = target
        waits = list(inst.sync_info.on_wait)
        updates = list(inst.sync_info.on_update or [])
        chunks = [waits[i:i + maxw] for i in range(0, len(waits), maxw)]
        nops = []
        for ch in chunks[:-1]:
            bnop = nc.engines[inst.engine].nop(nofuse=True, hint="waitsplit")
            nop_inst = _remove_by_name(bnop.ins.name)
            nop_inst.sync_info = mybir.SyncInfo(on_wait=ch, on_update=[])
            nops.append(nop_inst)
        inst.sync_info = mybir.SyncInfo(on_wait=chunks[-1], on_update=updates)
        lst = bb.instructions
        idx = next(i for i, x in enumerate(lst) if x.name == name)
        lst[idx:idx] = nops
        bb.instructions = lst
    return len(offenders)


# ------------------------------------------------------------------- constants
B, S, D = 1, 2048, 4096
H, G = 32, 8
HD = D // H            # 128
NC = 8                 # cores
HPC = H // NC          # heads per core = 4
OC = D // NC           # out columns per core = 512
P = 128
KT = D // P            # 32 contraction tiles
SCH = 256              # sequence chunk width for projections/attention
NSC = S // SCH         # 8
NKB = S // P           # 16 key tiles
SCALE = float(1.0 / np.sqrt(np.float32(HD)))

f32 = mybir.dt.float32
f32r = mybir.dt.float32r
bf16 = mybir.dt.bfloat16

Copy = mybir.ActivationFunctionType.Copy
Exp = mybir.ActivationFunctionType.Exp


def _build_program():
    nc = bass.Bass()
    xT = nc.declare_dram_parameter("xT", [P, KT, S], f32r, isOutput=False)
    wq = nc.declare_dram_parameter("wq", [P, KT, OC], f32r, isOutput=False)
    wk = nc.declare_dram_parameter("wk", [P, KT, HD], f32r, isOutput=False)
    wv = nc.declare_dram_parameter("wv", [P, KT, HD], f32r, isOutput=False)
    wo = nc.declare_dram_parameter("wo", [P, KT, OC], f32r, isOutput=False)
    cosT = nc.declare_dram_parameter("cosT", [HD, S], f32, isOutput=False)
    sinT = nc.declare_dram_parameter("sinT", [HD, S], f32, isOutput=False)
    tri = nc.declare_dram_parameter("tri", [P, P], f32, isOutput=False)
    out = nc.declare_dram_parameter("out", [S, OC], f32, isOutput=True)

    cc_in = nc.dram_tensor("cc_in", [HPC * HD, S], f32r)
    cc_out = nc.dram_tensor("cc_out", [D, S], f32r, addr_space="Shared")

    with tile.TileContext(nc) as tc:
        with (
            tc.tile_pool(name="singles", bufs=1) as singles,
            tc.tile_pool(name="wbig", bufs=1) as wbig,
            tc.tile_pool(name="stream", bufs=36) as stream,
            tc.tile_pool(name="qts", bufs=8) as qtsp,
            tc.tile_pool(name="pt", bufs=18) as ptp,
            tc.tile_pool(name="work", bufs=6) as work,
            tc.tile_pool(name="evict", bufs=4) as evictp,
            tc.tile_pool(name="ps", bufs=1, space="PSUM") as psp,
        ):
            # ---- constants / persistents
            ident = singles.tile([P, P], f32)
            make_identity(nc, ident[:])
            trim = singles.tile([P, P], bf16)
            tri_sb = singles.tile([P, P], f32)
            nc.sync.dma_start(out=tri_sb[:], in_=tri[:])
            nc.vector.tensor_copy(trim[:], tri_sb[:])

            cos_sb = singles.tile([HD, S], f32)
            nc.sync.dma_start(out=cos_sb[:], in_=cosT[:])
            sin_sb = singles.tile([HD, S], f32)
            nc.sync.dma_start(out=sin_sb[:], in_=sinT[:])

            wq_sb = wbig.tile([P, KT, OC], f32r, tag="wbig")
            nc.sync.dma_start(out=wq_sb[:], in_=wq[:])
            wk_sb = singles.tile([P, KT, HD], f32r)
            nc.sync.dma_start(out=wk_sb[:], in_=wk[:])
            wv_sb = singles.tile([P, KT, HD], f32r)
            nc.sync.dma_start(out=wv_sb[:], in_=wv[:])

            kT_all = singles.tile([HD, S], f32r)
            vT_all = singles.tile([HD, S], f32)
            vp_all = singles.tile([P, NKB, HD + 1], bf16)
            nc.vector.memset(vp_all[:], 1.0)

            def rope_evict(ps_t, dst, dst0, tab0):
                """ps_t: PSUM [HD, SCH] pre-rope; writes dst[:, dst0:dst0+SCH]
                (f32r) using rope tables at absolute position tab0."""
                rot = work.tile([HD, SCH], f32, tag="rot", bufs=3)
                nc.scalar.activation(out=rot[0:64, :], in_=ps_t[64:128, :],
                                     func=Copy, scale=-1.0)
                nc.scalar.activation(out=rot[64:128, :], in_=ps_t[0:64, :],
                                     func=Copy)
                m1 = work.tile([HD, SCH], f32, tag="m1", bufs=3)
                nc.vector.tensor_mul(m1[:], ps_t[:], cos_sb[:, tab0:tab0 + SCH])
                nc.vector.tensor_mul(rot[:], rot[:], sin_sb[:, tab0:tab0 + SCH])
                nc.vector.tensor_add(dst[:, dst0:dst0 + SCH], m1[:], rot[:])

            # ================= phase 1: projections + attention, per s-chunk
            for sc in range(NSC):
                s0 = sc * SCH
                xts = []
                for kt in range(KT):
                    t = stream.tile([P, SCH], f32r, tag="stream")
                    nc.sync.dma_start(out=t[:], in_=xT[:, kt, s0:s0 + SCH])
                    xts.append(t)

                # K projection -> RoPE -> kT_all
                ps_k = psp.tile([P, SCH], f32, tag="a", bufs=3)
                for kt in range(KT):
                    nc.tensor.matmul(ps_k[:], wk_sb[:, kt, :], xts[kt][:],
                                     start=(kt == 0), stop=(kt == KT - 1))
                rope_evict(ps_k, kT_all, s0, s0)

                # V projection (transposed) -> vT_all
                ps_v = psp.tile([P, SCH], f32, tag="a", bufs=3)
                for kt in range(KT):
                    nc.tensor.matmul(ps_v[:], wv_sb[:, kt, :], xts[kt][:],
                                     start=(kt == 0), stop=(kt == KT - 1))
                nc.scalar.copy(vT_all[:, s0:s0 + SCH], ps_v[:])

                # transpose vT chunk into natural-layout vp_all tiles
                for half in range(SCH // P):
                    kb = sc * (SCH // P) + half
                    ps_vt = psp.tile([P, P], f32, tag="b", bufs=2)
                    nc.tensor.transpose(
                        ps_vt[:], vT_all[:, kb * P:(kb + 1) * P], ident[:]
                    )
                    nc.scalar.copy(vp_all[:, kb, 0:HD], ps_vt[:])

                # Q projections + RoPE (4 heads)
                qts = []
                for h in range(HPC):
                    ps_q = psp.tile([P, SCH], f32, tag="a", bufs=3)
                    for kt in range(KT):
                        nc.tensor.matmul(
                            ps_q[:], wq_sb[:, kt, h * P:(h + 1) * P], xts[kt][:],
                            start=(kt == 0), stop=(kt == KT - 1))
                    qt = qtsp.tile([HD, SCH], f32r, tag="qts")
                    rope_evict(ps_q, qt, 0, s0)
                    qts.append(qt)

                # attention for this chunk's queries
                nq = SCH // P  # q-subchunks of 128
                for h in range(HPC):
                    pts = []
                    for kb in range(nq * sc + nq):
                        ps_s = psp.tile([P, SCH], f32, tag="a", bufs=3)
                        nc.tensor.matmul(ps_s[:], kT_all[:, kb * P:(kb + 1) * P],
                                         qts[h][:], start=True, stop=True)
                        pt = ptp.tile([P, SCH], bf16, tag="pt")
                        nc.scalar.activation(out=pt[:], in_=ps_s[:],
                                             func=Exp, scale=SCALE)
                        diag = kb - nq * sc
                        if 0 <= diag < nq:
                            nc.vector.tensor_mul(
                                pt[:, diag * P:(diag + 1) * P],
                                pt[:, diag * P:(diag + 1) * P], trim[:])
                        pts.append(pt)

                    for qh in range(nq):
                        iqc = nq * sc + qh
                        ps_c = psp.tile([P, HD + 1], f32, tag="b", bufs=2)
                        for kb in range(iqc + 1):
                            nc.tensor.matmul(
                                ps_c[:], pts[kb][:, qh * P:(qh + 1) * P],
                                vp_all[:, kb, :],
                                start=(kb == 0), stop=(kb == iqc))
                        rden = work.tile([P, 1], f32, tag="rden", bufs=4)
                        nc.vector.reciprocal(rden[:], ps_c[:, HD:HD + 1])
                        ctxn = work.tile([P, HD], f32, tag="ctxn", bufs=4)
                        nc.vector.tensor_scalar_mul(ctxn[:], ps_c[:, 0:HD],
                                                    rden[:])
                        ps_t = psp.tile([P, P], f32, tag="b", bufs=2)
                        nc.tensor.transpose(ps_t[:], ctxn[:], ident[:])
                        ctxT_sb = evictp.tile([HD, P], f32r, tag="ctxT")
                        nc.scalar.copy(ctxT_sb[:], ps_t[:])
                        nc.sync.dma_start(
                            out=cc_in[h * HD:(h + 1) * HD, iqc * P:(iqc + 1) * P],
                            in_=ctxT_sb[:])

            # ================= phase 2: AllGather + Wo
            nc.gpsimd.collective_compute(
                "AllGather",
                mybir.AluOpType.bypass,
                replica_groups=[list(range(NC))],
                ins=[cc_in[:]],
                outs=[cc_out[:]],
            )

            wo_sb = wbig.tile([P, KT, OC], f32r, tag="wbig")
            nc.sync.dma_start(out=wo_sb[:], in_=wo[:])

            cc3 = cc_out[:].rearrange("(t p) s -> p t s", p=P)
            for sq in range(S // P):
                ccts = []
                for dt2 in range(KT // 2):
                    t = stream.tile([P, 2, P], f32r, tag="stream")
                    nc.sync.dma_start(
                        out=t[:],
                        in_=cc3[:, 2 * dt2:2 * dt2 + 2, sq * P:(sq + 1) * P])
                    ccts.append(t)
                ps_o = psp.tile([P, OC], f32, tag="c", bufs=1)
                for kt in range(KT):
                    nc.tensor.matmul(
                        ps_o[:], ccts[kt // 2][:, kt % 2, :], wo_sb[:, kt, :],
                        start=(kt == 0), stop=(kt == KT - 1))
                out_sb = evictp.tile([P, OC], f32, tag="osb", bufs=2)
                nc.scalar.copy(out_sb[:], ps_o[:])
                nc.sync.dma_start(out=out[sq * P:(sq + 1) * P, :], in_=out_sb[:])

    return nc


_PROGRAM_CACHE = {}


def _get_program():
    if "nc" not in _PROGRAM_CACHE:
        nc = _build_program()
        _split_multi_waits(nc, maxw=1)
        _PROGRAM_CACHE["nc"] = nc
    return _PROGRAM_CACHE["nc"]


def _rope_tables_T():
    inv_freq = (1.0 / (10000.0 ** (np.arange(0, HD, 2, dtype=np.float32) / HD))
                ).astype(np.float32)
    ang = np.arange(S, dtype=np.float32)[:, None] * inv_freq[None, :]
    ang = np.concatenate([ang, ang], axis=-1)  # [S, HD]
    return (np.ascontiguousarray(np.cos(ang).T.astype(np.float32)),
            np.ascontiguousarray(np.sin(ang).T.astype(np.float32)))


def _prep_in_maps(x, Wq, Wk, Wv, Wo):
    x2d = np.asarray(x, np.float32).reshape(S, D)
    xT_dev = np.ascontiguousarray(x2d.T.reshape(KT, P, S).transpose(1, 0, 2))
    cosT, sinT = _rope_tables_T()
    tri_np = (np.arange(P)[:, None] <= np.arange(P)[None, :]).astype(np.float32)

    def wtiles(Wslice, width):
        return np.ascontiguousarray(
            np.asarray(Wslice, np.float32).reshape(KT, P, width)
            .transpose(1, 0, 2))

    in_maps = []
    for c in range(NC):
        in_maps.append({
            "xT": xT_dev,
            "wq": wtiles(Wq[:, c * OC:(c + 1) * OC], OC),
            "wk": wtiles(Wk[:, c * HD:(c + 1) * HD], HD),
            "wv": wtiles(Wv[:, c * HD:(c + 1) * HD], HD),
            "wo": wtiles(Wo[:, c * OC:(c + 1) * OC], OC),
            "cosT": cosT,
            "sinT": sinT,
            "tri": tri_np,
        })
    return in_maps


def _run(inputs, trace=False):
    nc = _get_program()
    in_maps = _prep_in_maps(inputs["x"], inputs["Wq"], inputs["Wk"],
                            inputs["Wv"], inputs["Wo"])
    res = run_bass_kernel_spmd(nc, in_maps, core_ids=list(range(NC)),
                               trace=trace)
    out = np.concatenate([res.results[c]["out"] for c in range(NC)], axis=1)
    return out.reshape(B, S, D).astype(np.float32), res


def kernel(**inputs):
    out, _ = _run(inputs, trace=False)
    return out


# revision 5
# speedup vs baseline: 1.3484x; 1.3484x over previous
"""GQA (B=1, S=2048, D=4096, H=32, G=8) on 8 TRN2 NeuronCores.

Sharding: tensor-parallel over heads — core c owns query heads 4c..4c+3 and
KV group c. Per core: qT/kT/vT projections from full x (transposed layouts),
RoPE, causal attention with transposed probs (exp without max-subtraction —
scores are bounded; denominator via a ones-column in V'), normalization folded
into a per-partition scalar multiply, PE-transpose of ctx, AllGather of ctxT
(4MB/rank), then a column-sharded Wo matmul. Host slices weights / transposes
x / concatenates output columns.

Self-contained: no sibling imports; hardcoded shapes.
"""
import contextlib
import ctypes
import os
import sys
import types

import numpy as np

os.environ.setdefault("MYCRO_LOCAL_CACHE", "1")

for _p in ("/opt/trn_rl_repo", "/root/.axon_site/_ro/trn_rl_repo"):
    if _p not in sys.path and os.path.isdir(_p):
        sys.path.append(_p)

import concourse.bass as bass
import concourse.tile as tile
from concourse import mybir
from concourse.bass_utils import run_bass_kernel_spmd
from concourse.masks import make_identity

# ---------------------------------------------------------------- profiling shim
_SO_PATH = "/opt/axon/libaxon_pjrt.so"
_hook_holder = [None]


def _ntff_profile_via_ctypes(so_path):
    try:
        lib = ctypes.CDLL(so_path)
    except OSError:
        return None
    if not hasattr(lib, "axon_start_nrt_profile"):
        return None
    lib.axon_start_nrt_profile.argtypes = [
        ctypes.POINTER(ctypes.c_int64),
        ctypes.c_size_t,
    ]
    lib.axon_start_nrt_profile.restype = ctypes.c_int64
    lib.axon_stop_nrt_profile.argtypes = [ctypes.c_char_p]
    lib.axon_stop_nrt_profile.restype = ctypes.c_int64

    @contextlib.contextmanager
    def _hook(output_dir, device_ids):
        import jax

        jax.devices()
        if device_ids:
            ids = (ctypes.c_int64 * len(device_ids))(*device_ids)
            rc = lib.axon_start_nrt_profile(ids, len(device_ids))
        else:
            rc = lib.axon_start_nrt_profile(None, 0)
        if rc != 0:
            raise RuntimeError(f"axon_start_nrt_profile rc={rc}")
        try:
            yield
        finally:
            n = lib.axon_stop_nrt_profile(str(output_dir).encode())
            if n <= 0:
                print(f"WARNING: ntff capture wrote {n} files", file=sys.stderr)

    return _hook


def _install_prof_shim():
    if "antenv.axon_hooks" not in sys.modules:
        mod = types.ModuleType("antenv.axon_hooks")
        mod.set_axon_ntff_profile_hook = lambda h: _hook_holder.__setitem__(0, h)
        mod.get_axon_ntff_profile_hook = lambda: _hook_holder[0]
        sys.modules["antenv.axon_hooks"] = mod
    _hook_holder[0] = _ntff_profile_via_ctypes(_SO_PATH)
    import concourse.bass_utils as bu

    bu.upload_artifacts = lambda tmpdir: tmpdir


_install_prof_shim()

# ------------------------------------------------------------- wait-split pass
def _split_multi_waits(nc, maxw=1):
    """walrus in this container allows only one sync-wait per instruction;
    split extras onto nops inserted before the offender (same engine/block)."""

    def _remove_by_name(name):
        for f in nc.m.functions:
            for bb in f.blocks:
                for i, inst in enumerate(bb.instructions):
                    if inst.name == name:
                        lst = bb.instructions
                        del lst[i]
                        bb.instructions = lst
                        return inst
        raise KeyError(name)

    offenders = []
    for f in nc.m.functions:
        for bb in f.blocks:
            for inst in bb.instructions:
                si = inst.sync_info
                if si and si.on_wait and len(si.on_wait) > maxw:
                    offenders.append(inst.name)
    for name in offenders:
        target = None
        for f in nc.m.functions:
            for bb in f.blocks:
                for idx, inst in enumerate(bb.instructions):
                    if inst.name == name:
                        target = (bb, inst)
                        break
                if target:
                    break
            if target:
                break
        bb, inst = target
        waits = list(inst.sync_info.on_wait)
        updates = list(inst.sync_info.on_update or [])
        chunks = [waits[i:i + maxw] for i in range(0, len(waits), maxw)]
        nops = []
        for ch in chunks[:-1]:
            bnop = nc.engines[inst.engine].nop(nofuse=True, hint="waitsplit")
            nop_inst = _remove_by_name(bnop.ins.name)
            nop_inst.sync_info = mybir.SyncInfo(on_wait=ch, on_update=[])
            nops.append(nop_inst)
        inst.sync_info = mybir.SyncInfo(on_wait=chunks[-1], on_update=updates)
        lst = bb.instructions
        idx = next(i for i, x in enumerate(lst) if x.name == name)
        lst[idx:idx] = nops
        bb.instructions = lst
    return len(offenders)


# ------------------------------------------------------------------- constants
B, S, D = 1, 2048, 4096
H, G = 32, 8
HD = D // H            # 128
NC = 8                 # cores
HPC = H // NC          # heads per core = 4
OC = D // NC           # out columns per core = 512
P = 128
KT = D // P            # 32 contraction tiles
SCH = 256              # sequence chunk width for projections/attention
NSC = S // SCH         # 8
NKB = S // P           # 16 key tiles
SCALE = float(1.0 / np.sqrt(np.float32(HD)))

f32 = mybir.dt.float32
f32r = mybir.dt.float32r
bf16 = mybir.dt.bfloat16

Copy = mybir.ActivationFunctionType.Copy
Exp = mybir.ActivationFunctionType.Exp


def _build_program():
    nc = bass.Bass()
    xT = nc.declare_dram_parameter("xT", [P, KT, S], f32r, isOutput=False)
    wq = nc.declare_dram_parameter("wq", [P, KT, OC], f32r, isOutput=False)
    wk = nc.declare_dram_parameter("wk", [P, KT, HD], f32r, isOutput=False)
    wv = nc.declare_dram_parameter("wv", [P, KT, HD], f32r, isOutput=False)
    wo = nc.declare_dram_parameter("wo", [P, KT, OC], f32r, isOutput=False)
    cosT = nc.declare_dram_parameter("cosT", [HD, S], f32, isOutput=False)
    sinT = nc.declare_dram_parameter("sinT", [HD, S], f32, isOutput=False)
    tri = nc.declare_dram_parameter("tri", [P, P], f32, isOutput=False)
    out = nc.declare_dram_parameter("out", [S, OC], f32, isOutput=True)

    NSPL = 4
    SPW = S // NSPL  # 512 columns per collective split
    cc_ins = [nc.dram_tensor(f"cc_in{k}", [HPC * HD, SPW], f32r)
              for k in range(NSPL)]
    cc_outs = [nc.dram_tensor(f"cc_out{k}", [D, SPW], f32r, addr_space="Shared")
               for k in range(NSPL)]

    with tile.TileContext(nc) as tc:
        with (
            tc.tile_pool(name="singles", bufs=1) as singles,
            tc.tile_pool(name="wbig", bufs=1) as wbig,
            tc.tile_pool(name="stream", bufs=6) as stream,
            tc.tile_pool(name="qts", bufs=6) as qtsp,
            tc.tile_pool(name="pt", bufs=17) as ptp,
            tc.tile_pool(name="work", bufs=6) as work,
            tc.tile_pool(name="evict", bufs=4) as evictp,
            tc.tile_pool(name="ps", bufs=1, space="PSUM") as psp,
        ):
            # ---- constants / persistents
            ident = singles.tile([P, P], f32)
            make_identity(nc, ident[:])
            trim = singles.tile([P, P], bf16)
            tri_sb = singles.tile([P, P], f32)
            nc.sync.dma_start(out=tri_sb[:], in_=tri[:])
            nc.vector.tensor_copy(trim[:], tri_sb[:])

            cos_sb = singles.tile([HD, S], f32)
            nc.sync.dma_start(out=cos_sb[:], in_=cosT[:])
            sin_sb = singles.tile([HD, S], f32)
            nc.sync.dma_start(out=sin_sb[:], in_=sinT[:])

            wq_sb = wbig.tile([P, KT, OC], f32r, tag="wbig")
            nc.sync.dma_start(out=wq_sb[:], in_=wq[:])
            wk_sb = singles.tile([P, KT, HD], f32r)
            nc.sync.dma_start(out=wk_sb[:], in_=wk[:])
            wv_sb = singles.tile([P, KT, HD], f32r)
            nc.sync.dma_start(out=wv_sb[:], in_=wv[:])

            kT_all = singles.tile([HD, S], f32r)
            vp_all = singles.tile([P, NKB, HD + 1], bf16)
            nc.vector.memset(vp_all[:], 1.0)

            def rope_evict(ps_t, dst, dst0, tab0):
                """ps_t: PSUM [HD, SCH] pre-rope; writes dst[:, dst0:dst0+SCH]
                (f32r) using rope tables at absolute position tab0."""
                rot = work.tile([HD, SCH], f32, tag="rot", bufs=2)
                nc.scalar.activation(out=rot[0:64, :], in_=ps_t[64:128, :],
                                     func=Copy, scale=-1.0)
                nc.scalar.activation(out=rot[64:128, :], in_=ps_t[0:64, :],
                                     func=Copy)
                m1 = work.tile([HD, SCH], f32, tag="m1", bufs=2)
                nc.vector.tensor_mul(m1[:], ps_t[:], cos_sb[:, tab0:tab0 + SCH])
                nc.vector.tensor_mul(rot[:], rot[:], sin_sb[:, tab0:tab0 + SCH])
                nc.vector.tensor_add(dst[:, dst0:dst0 + SCH], m1[:], rot[:])

            # ================= phase 1: projections + attention, per s-chunk
            for sc in range(NSC):
                s0 = sc * SCH
                xtg = []
                for g in range(KT // 8):
                    t = stream.tile([P, 8, SCH], f32r, tag="stream", bufs=6)
                    nc.sync.dma_start(out=t[:],
                                      in_=xT[:, 8 * g:8 * g + 8, s0:s0 + SCH])
                    xtg.append(t)

                def xts(kt):
                    return xtg[kt // 8][:, kt % 8, :]

                # K projection -> RoPE -> kT_all
                ps_k = psp.tile([P, SCH], f32, tag="a", bufs=3)
                for kt in range(KT):
                    nc.tensor.matmul(ps_k[:], wk_sb[:, kt, :], xts(kt),
                                     start=(kt == 0), stop=(kt == KT - 1))
                rope_evict(ps_k, kT_all, s0, s0)

                # V projection (transposed) -> vT_all
                ps_v = psp.tile([P, SCH], f32, tag="a", bufs=3)
                for kt in range(KT):
                    nc.tensor.matmul(ps_v[:], wv_sb[:, kt, :], xts(kt),
                                     start=(kt == 0), stop=(kt == KT - 1))
                vc = work.tile([HD, SCH], f32, tag="vc", bufs=2)
                nc.scalar.copy(vc[:], ps_v[:])
                for half in range(SCH // P):
                    kb = sc * (SCH // P) + half
                    ps_vt = psp.tile([P, P], f32, tag="b", bufs=2)
                    nc.tensor.transpose(
                        ps_vt[:], vc[:, half * P:(half + 1) * P], ident[:]
                    )
                    nc.scalar.copy(vp_all[:, kb, 0:HD], ps_vt[:])

                # Q projections + RoPE (4 heads)
                qts = []
                for h in range(HPC):
                    ps_q = psp.tile([P, SCH], f32, tag="a", bufs=3)
                    for kt in range(KT):
                        nc.tensor.matmul(
                            ps_q[:], wq_sb[:, kt, h * P:(h + 1) * P], xts(kt),
                            start=(kt == 0), stop=(kt == KT - 1))
                    qt = qtsp.tile([HD, SCH], f32r, tag="qts")
                    rope_evict(ps_q, qt, 0, s0)
                    qts.append(qt)

                # attention for this chunk's queries
                nq = SCH // P  # q-subchunks of 128
                for h in range(HPC):
                    pts = []
                    for kb in range(nq * sc + nq):
                        ps_s = psp.tile([P, SCH], f32, tag="a", bufs=3)
                        nc.tensor.matmul(ps_s[:], kT_all[:, kb * P:(kb + 1) * P],
                                         qts[h][:], start=True, stop=True)
                        pt = ptp.tile([P, SCH], bf16, tag="pt")
                        nc.scalar.activation(out=pt[:], in_=ps_s[:],
                                             func=Exp, scale=SCALE)
                        diag = kb - nq * sc
                        if 0 <= diag < nq:
                            nc.vector.tensor_mul(
                                pt[:, diag * P:(diag + 1) * P],
                                pt[:, diag * P:(diag + 1) * P], trim[:])
                        pts.append(pt)

                    for qh in range(nq):
                        iqc = nq * sc + qh
                        ps_c = psp.tile([P, HD + 1], f32, tag="b", bufs=2)
                        for kb in range(iqc + 1):
                            nc.tensor.matmul(
                                ps_c[:], pts[kb][:, qh * P:(qh + 1) * P],
                                vp_all[:, kb, :],
                                start=(kb == 0), stop=(kb == iqc))
                        rden = work.tile([P, 1], f32, tag="rden", bufs=4)
                        nc.vector.reciprocal(rden[:], ps_c[:, HD:HD + 1])
                        ctxn = work.tile([P, HD], f32, tag="ctxn", bufs=4)
                        nc.vector.tensor_scalar_mul(ctxn[:], ps_c[:, 0:HD],
                                                    rden[:])
                        ps_t = psp.tile([P, P], f32, tag="b", bufs=2)
                        nc.tensor.transpose(ps_t[:], ctxn[:], ident[:])
                        ctxT_sb = evictp.tile([HD, P], f32r, tag="ctxT", bufs=3)
                        nc.scalar.copy(ctxT_sb[:], ps_t[:])
                        spl, lc = iqc // 4, iqc % 4
                        nc.sync.dma_start(
                            out=cc_ins[spl][h * HD:(h + 1) * HD,
                                            lc * P:(lc + 1) * P],
                            in_=ctxT_sb[:])

                if sc % 2 == 1:
                    k = (sc - 1) // 2
                    nc.gpsimd.collective_compute(
                        "AllGather",
                        mybir.AluOpType.bypass,
                        replica_groups=[list(range(NC))],
                        ins=[cc_ins[k][:]],
                        outs=[cc_outs[k][:]],
                    )

            # ================= phase 2: Wo (collectives were issued in-loop)
            wo_sb = wbig.tile([P, KT, OC], f32r, tag="wbig")
            nc.sync.dma_start(out=wo_sb[:], in_=wo[:])

            cc3s = [cc_outs[k][:].rearrange("(t p) s -> p t s", p=P)
                    for k in range(NSPL)]
            for sq in range(S // P):
                spl, ls = sq // 4, sq % 4
                ccts = []
                for g in range(2):
                    t = stream.tile([P, 16, P], f32r, tag="stream", bufs=6)
                    nc.sync.dma_start(
                        out=t[:],
                        in_=cc3s[spl][:, 16 * g:16 * g + 16,
                                      ls * P:(ls + 1) * P])
                    ccts.append(t)
                ps_o = psp.tile([P, OC], f32, tag="c", bufs=1)
                for kt in range(KT):
                    nc.tensor.matmul(
                        ps_o[:], ccts[kt // 16][:, kt % 16, :], wo_sb[:, kt, :],
                        start=(kt == 0), stop=(kt == KT - 1))
                out_sb = evictp.tile([P, OC], f32, tag="osb", bufs=2)
                nc.scalar.copy(out_sb[:], ps_o[:])
                nc.sync.dma_start(out=out[sq * P:(sq + 1) * P, :], in_=out_sb[:])

    return nc


_PROGRAM_CACHE = {}


def _get_program():
    if "nc" not in _PROGRAM_CACHE:
        nc = _build_program()
        _split_multi_waits(nc, maxw=1)
        _PROGRAM_CACHE["nc"] = nc
    return _PROGRAM_CACHE["nc"]


def _rope_tables_T():
    inv_freq = (1.0 / (10000.0 ** (np.arange(0, HD, 2, dtype=np.float32) / HD))
                ).astype(np.float32)
    ang = np.arange(S, dtype=np.float32)[:, None] * inv_freq[None, :]
    ang = np.concatenate([ang, ang], axis=-1)  # [S, HD]
    return (np.ascontiguousarray(np.cos(ang).T.astype(np.float32)),
            np.ascontiguousarray(np.sin(ang).T.astype(np.float32)))


def _prep_in_maps(x, Wq, Wk, Wv, Wo):
    x2d = np.asarray(x, np.float32).reshape(S, D)
    xT_dev = np.ascontiguousarray(x2d.T.reshape(KT, P, S).transpose(1, 0, 2))
    cosT, sinT = _rope_tables_T()
    tri_np = (np.arange(P)[:, None] <= np.arange(P)[None, :]).astype(np.float32)

    def wtiles(Wslice, width):
        return np.ascontiguousarray(
            np.asarray(Wslice, np.float32).reshape(KT, P, width)
            .transpose(1, 0, 2))

    in_maps = []
    for c in range(NC):
        in_maps.append({
            "xT": xT_dev,
            "wq": wtiles(Wq[:, c * OC:(c + 1) * OC], OC),
            "wk": wtiles(Wk[:, c * HD:(c + 1) * HD], HD),
            "wv": wtiles(Wv[:, c * HD:(c + 1) * HD], HD),
            "wo": wtiles(Wo[:, c * OC:(c + 1) * OC], OC),
            "cosT": cosT,
            "sinT": sinT,
            "tri": tri_np,
        })
    return in_maps


def _run(inputs, trace=False):
    nc = _get_program()
    in_maps = _prep_in_maps(inputs["x"], inputs["Wq"], inputs["Wk"],
                            inputs["Wv"], inputs["Wo"])
    res = run_bass_kernel_spmd(nc, in_maps, core_ids=list(range(NC)),
                               trace=trace)
    out = np.concatenate([res.results[c]["out"] for c in range(NC)], axis=1)
    return out.reshape(B, S, D).astype(np.float32), res


def kernel(**inputs):
    out, _ = _run(inputs, trace=False)
    return out
